# revision 1
# baseline (speedup 1.0000x reference)
"""GPT-2-small (B=4,T=1024,C=768,H=12,L=2,V=50257) forward, last-token logits.

Sharding: core c handles batch b=c//2 (body replicated within each core pair)
and vocab shard c of the tied lm_head. One tiny AllGather exchanges the four
final-LN last-token vectors so every core can compute all 4 batches against
its own vocab shard. Matmuls run in bf16 (fp32 accumulate in PSUM).

Layout strategy: activations channel-major ("T" = transposed [C,tok]) feed
the PE as stationary/moving operands without any transposes inside attention;
scores are computed k-major (ST layout) so softmax sums use a ones-matmul and
exp runs on the scalar engine. Only layer-norm outputs are transposed
(PE transpose via identity). Layer 2 computes K/V for all tokens but runs
attention/MLP only for the last 128-token block (logits need only token 1023).
"""
import sys, os
sys.path.insert(0, "/opt/trn_rl_repo")
sys.path.insert(0, os.path.dirname(os.path.abspath(__file__)))
import numpy as np
import ml_dtypes
from concourse import bass, mybir
import concourse.tile as tile
from concourse.bass_utils import run_bass_kernel_spmd

# ---- inlined walrus wait-limit workaround (was tile_patch.py) ----
# The tail drain and DMA pseudo-instructions may carry only ~1 sem-wait each
# under this walrus build; split excess waits onto same-engine NOPs.
import concourse.tile as tile_mod
from concourse.tile import ScopedClock

MAXW = 1



def _patched_drain_and_barrier(self, tick_clock, wait_clock):
    nc = self.nc
    drain_inst = nc.sync.drain()
    wait_clock.add_sem_waits(
        drain_inst.ins, ScopedClock({None: tick_clock.global_clock})
    )
    si = drain_inst.ins.sync_info
    if si is not None and si.on_wait and len(si.on_wait) > MAXW:
        waits = list(si.on_wait)
        drain_inst.ins.sync_info = mybir.SyncInfo(
            on_wait=waits[:MAXW], on_update=list(si.on_update or [])
        )
        rest = waits[MAXW:]
        while rest:
            nop = nc.sync.nop()
            nop.ins.sync_info = mybir.SyncInfo(on_wait=rest[:MAXW], on_update=[])
            rest = rest[MAXW:]

    nc.all_engine_barrier()
    assert self.sems is not None
    popped = nc._tile_sem_poison_stack.pop()
    assert popped is self._sem_poison
    nc.clear_and_free_semaphores(list(self.sems.allocated().values()))
    nc.all_engine_barrier()


tile_mod.TileContext._drain_and_barrier = _patched_drain_and_barrier

_DMA_LIKE = ("DMA", "Collective", "Memset")
_ctr = [0]


def split_excess_waits(nc):
    """Walrus allows only 1 sem-wait on DMA pseudo-instructions and ~8 on
    regular engine instructions. Move excess waits onto same-engine NOPs
    inserted immediately before the offending instruction."""
    import bass_rust

    for f in nc.m.functions:
        for b in f.blocks:
            il = b.instructions
            i = 0
            while i < len(il):
                inst = il[i]
                si = getattr(inst, "sync_info", None)
                waits = list(si.on_wait) if (si is not None and si.on_wait) else []
                opc = str(getattr(inst, "opcode", ""))
                limit = 1 if any(k in opc for k in _DMA_LIKE) else MAXW
                if len(waits) > limit:
                    keep = waits[-limit:] if limit > 0 else []
                    extra = waits[: len(waits) - limit]
                    inst.sync_info = mybir.SyncInfo(
                        on_wait=keep, on_update=list(si.on_update or [])
                    )
                    while extra:
                        chunk, extra = extra[:MAXW], extra[MAXW:]
                        nop = bass_rust.InstNoOp(
                            name=f"wsplit-{_ctr[0]}", ins=[], outs=[]
                        )
                        _ctr[0] += 1
                        nop.engine = inst.engine
                        nop.sync_info = mybir.SyncInfo(on_wait=chunk, on_update=[])
                        il.insert(i, nop)
                        i += 1
                i += 1

P = 128
B, T, C, H, L = 4, 1024, 768, 12, 2
DH = C // H          # 64
HID = 4 * C          # 3072
V = 50257
NT = T // P          # 8 token blocks
KC = C // P          # 6 channel chunks
NHID = HID // P      # 24
VS = 6656            # vocab shard per core (8*6656 = 53248 >= V)
VP = 8 * VS
EPS = 1e-5
BF = mybir.dt.bfloat16
F32 = mybir.dt.float32
AF = mybir.ActivationFunctionType
OP = mybir.AluOpType
NCORES = 8

_cache = {}


def _build():
    nc = bass.Bass()
    idx_d = nc.dram_tensor("idx", [NT, P], mybir.dt.int32, kind="ExternalInput")
    wte_d = nc.dram_tensor("wte", [VP, C], BF, kind="ExternalInput")
    wlm_d = nc.dram_tensor("wlm", [C, VS], BF, kind="ExternalInput")
    wpe_d = nc.dram_tensor("wpe", [T, C], F32, kind="ExternalInput")
    wqkv_d = nc.dram_tensor("wqkv", [L, C, 3 * C], BF, kind="ExternalInput")
    wproj_d = nc.dram_tensor("wproj", [L, C, C], BF, kind="ExternalInput")
    wfc_d = nc.dram_tensor("wfc", [L, C, HID], BF, kind="ExternalInput")
    wfcp_d = nc.dram_tensor("wfcp", [L, HID, C], BF, kind="ExternalInput")
    masks_d = nc.dram_tensor("masks", [P, 896], BF, kind="ExternalInput")
    ones_d = nc.dram_tensor("ones", [P, P], BF, kind="ExternalInput")
    eye_d = nc.dram_tensor("eye", [P, P], BF, kind="ExternalInput")
    logits_d = nc.dram_tensor("logits", [B, VS], F32, kind="ExternalOutput")

    cc_in = nc.dram_tensor("cc_in", [1, C], F32)
    cc_out = nc.dram_tensor("cc_out", [NCORES, C], F32, addr_space="Shared")

    with tile.TileContext(nc) as tc:
        with (
            tc.tile_pool(name="const", bufs=1) as cp,
            tc.tile_pool(name="acts", bufs=1) as ap,
            tc.tile_pool(name="wstream", bufs=2) as wp,
            tc.tile_pool(name="wfcol", bufs=1) as wfp,
            tc.tile_pool(name="wfcpcol", bufs=1) as wfpp,
            tc.tile_pool(name="scratch", bufs=2) as sp,
            tc.tile_pool(name="est", bufs=2) as ep,
            tc.tile_pool(name="mlp", bufs=1) as mp,
            tc.tile_pool(name="pb", bufs=4, space="PSUM") as pb,
            tc.tile_pool(name="pw", bufs=2, space="PSUM") as pw,
        ):
            ones = cp.tile([P, P], BF, tag="ones", name="ones")
            zb = cp.tile([P, 1], F32, tag="zb", name="zb")
            nc.gpsimd.memset(zb[:], 0.0)
            eb = cp.tile([P, 1], F32, tag="eb", name="eb")
            nc.gpsimd.memset(eb[:], EPS)
            nc.sync.dma_start(ones[:], ones_d[:])
            eye = cp.tile([P, P], BF, tag="eye", name="eye")
            nc.sync.dma_start(eye[:], eye_d[:])
            masks = cp.tile([P, 896], BF, tag="masks", name="masks")
            nc.sync.dma_start(masks[:], masks_d[:])
            idx_sb = cp.tile([P, NT], mybir.dt.int32, tag="idx", name="idx")
            nc.sync.dma_start(idx_sb[:], idx_d[:].rearrange("t p -> p t"))

            # persistent activation tiles
            x = [ap.tile([P, C], F32, tag=f"x{t}", name=f"x{t}") for t in range(NT)]
            xnT = [ap.tile([P, T], BF, tag=f"xnT{c}", name=f"xnT{c}") for c in range(KC)]
            qkT = [ap.tile([P, T], BF, tag=f"qkT{d}", name=f"qkT{d}") for d in range(12)]
            vtok = [ap.tile([P, C], BF, tag=f"v{t}", name=f"v{t}") for t in range(NT)]
            yT = [ap.tile([P, T], BF, tag=f"yT{c}", name=f"yT{c}") for c in range(KC)]
            s1 = ap.tile([P, NT], F32, tag="s1", name="s1")
            s2 = ap.tile([P, NT], F32, tag="s2", name="s2")
            mean = ap.tile([P, NT], F32, tag="mean", name="mean")
            var = ap.tile([P, NT], F32, tag="var", name="var")
            rstd = ap.tile([P, NT], F32, tag="rstd", name="rstd")

            def layer_norm(blocks, transpose=True):
                """x[t] (f32) -> normalized in place; transposed bf16 -> xnT."""
                for t in blocks:
                    nc.vector.reduce_sum(
                        out=s1[:, t : t + 1], in_=x[t][:], axis=mybir.AxisListType.X
                    )
                    sq = sp.tile([P, C], F32, tag="lnsq", name="lnsq")
                    nc.scalar.activation(
                        sq[:], x[t][:], AF.Square, bias=zb[:, 0:1], accum_out=s2[:, t : t + 1]
                    )
                # stats for all blocks at once (extra cols harmless)
                nc.vector.tensor_scalar_mul(mean[:], s1[:], 1.0 / C)
                nc.vector.tensor_scalar_mul(var[:], s2[:], 1.0 / C)
                msq = sp.tile([P, NT], F32, tag="lnmsq", name="lnmsq")
                nc.vector.tensor_tensor(msq[:], mean[:], mean[:], op=OP.mult)
                nc.vector.tensor_tensor(var[:], var[:], msq[:], op=OP.subtract)
                std = sp.tile([P, NT], F32, tag="lnstd", name="lnstd")
                nc.scalar.activation(std[:], var[:], AF.Sqrt, bias=eb[:, 0:1])
                nc.vector.reciprocal(rstd[:], std[:])
                for t in blocks:
                    nc.vector.tensor_scalar(
                        out=x[t][:], in0=x[t][:],
                        scalar1=mean[:, t : t + 1], scalar2=rstd[:, t : t + 1],
                        op0=OP.subtract, op1=OP.mult,
                    )
                    if not transpose:
                        continue
                    xtmp = sp.tile([P, C], BF, tag="xtmp", name="xtmp")
                    nc.vector.tensor_copy(out=xtmp[:], in_=x[t][:])
                    for c in range(KC):
                        tp = pb.tile([P, P], BF, tag="ps", name="tp")
                        nc.tensor.transpose(
                            tp[:], xtmp[:, c * P : (c + 1) * P], eye[:]
                        )
                        nc.vector.tensor_copy(
                            out=xnT[c][:, t * P : (t + 1) * P], in_=tp[:]
                        )

            # ---- embedding ----
            for t in range(NT):
                g = sp.tile([P, C], BF, tag="emb", name="emb")
                nc.gpsimd.indirect_dma_start(
                    out=g[:], out_offset=None, in_=wte_d[:],
                    in_offset=bass.IndirectOffsetOnAxis(ap=idx_sb[:, t : t + 1], axis=0),
                )
                pe = sp.tile([P, C], F32, tag="wpe", name="wpe")
                nc.sync.dma_start(pe[:], wpe_d[t * P : (t + 1) * P, :])
                nc.vector.tensor_tensor(x[t][:], g[:], pe[:], op=OP.add)

            for l in range(L):
                last = l == L - 1
                TB = [7] if last else list(range(NT))

                layer_norm(range(NT))

                # ---- q/k channel-major ---- (layer2: q only for last block)
                for nB in range(3):  # qkv columns [nB*512, nB*512+512) = q,q,k? 0..1536
                    wcol = wp.tile([P, KC, 512], BF, tag="w512", name="w512")
                    nc.sync.dma_start(
                        wcol[:],
                        wqkv_d[l, :, nB * 512 : (nB + 1) * 512].rearrange(
                            "(c p) n -> p c n", p=P
                        ),
                    )
                    for dl in range(4):
                        d = nB * 4 + dl
                        is_q = d < 6
                        if last and is_q:
                            spans = [(7 * P, P)]  # q only for token block 7
                        else:
                            spans = [(0, 512), (512, 512)]
                        for (o, w) in spans:
                            ps = pb.tile([P, 512], F32, tag="ps", name="psb")
                            for c in range(KC):
                                nc.tensor.matmul(
                                    ps[:, :w],
                                    wcol[:, c, dl * P : (dl + 1) * P],
                                    xnT[c][:, o : o + w],
                                    start=(c == 0), stop=(c == KC - 1),
                                )
                            nc.vector.tensor_copy(qkT[d][:, o : o + w], ps[:, :w])

                # ---- v token-major ----
                wv = wp.tile([P, KC, C], BF, tag="w768", name="w768")
                nc.sync.dma_start(
                    wv[:], wqkv_d[l, :, 2 * C :].rearrange("(c p) n -> p c n", p=P)
                )
                for t in range(NT):
                    ps = pw.tile([P, C], F32, tag="psw", name="psw")
                    for c in range(KC):
                        nc.tensor.matmul(
                            ps[:, 0:512], xnT[c][:, t * P : (t + 1) * P],
                            wv[:, c, 0:512], start=(c == 0), stop=(c == KC - 1),
                        )
                        nc.tensor.matmul(
                            ps[:, 512:768], xnT[c][:, t * P : (t + 1) * P],
                            wv[:, c, 512:768], start=(c == 0), stop=(c == KC - 1),
                        )
                    nc.vector.tensor_copy(vtok[t][:], ps[:])

                # ---- attention ----
                if last:
                    qspans = [(7 * P, P, 8)]  # (col offset, width, n key blocks)
                else:
                    qspans = [(0, 512, 4), (512, 512, 8)]
                for h in range(H):
                    kt = qkT[6 + h // 2]
                    qt = qkT[h // 2]
                    po = (h % 2) * DH
                    for (qo, qw, nkb) in qspans:
                        den = pb.tile([P, 512], F32, tag="ps", name="den")
                        yp = pb.tile([P, 512], F32, tag="ps", name="yp")
                        for kb in range(nkb):
                            st = pb.tile([P, 512], F32, tag="ps", name="st")
                            nc.tensor.matmul(
                                st[:, :qw],
                                kt[po : po + DH, kb * P : (kb + 1) * P],
                                qt[po : po + DH, qo : qo + qw],
                                start=True, stop=True,
                            )
                            est = ep.tile([P, 512], BF, tag="est", name="est")
                            nc.scalar.activation(est[:, :qw], st[:, :qw], AF.Exp, bias=zb[:, 0:1])
                            # causal mask on diagonal-region key blocks.
                            # query chunk starts at token qo; key block kb is
                            # fully valid iff kb*128+127 <= qo+col for all cols,
                            # i.e. kb < qo//128. Otherwise mask j = kb - qo//128
                            # (mask j: valid iff 128*j + kp <= qcol).
                            jj = kb - qo // P
                            if jj >= 0:
                                mo = 384 - 128 * jj
                                nc.vector.tensor_tensor(
                                    est[:, :qw], est[:, :qw],
                                    masks[:, mo : mo + qw], op=OP.mult,
                                )
                            nc.tensor.matmul(
                                den[:, :qw], ones[:], est[:, :qw],
                                start=(kb == 0), stop=(kb == nkb - 1),
                            )
                            nc.tensor.matmul(
                                yp[:DH, :qw], vtok[kb][:, h * DH : (h + 1) * DH],
                                est[:, :qw], start=(kb == 0), stop=(kb == nkb - 1),
                            )
                        rec = ep.tile([DH, 512], F32, tag="rec", name="rec")
                        nc.vector.reciprocal(rec[:, :qw], den[:DH, :qw])
                        nc.vector.tensor_tensor(
                            yT[h // 2][po : po + DH, qo : qo + qw],
                            yp[:DH, :qw], rec[:, :qw], op=OP.mult,
                        )

                # ---- attn proj + residual ----
                wpr = wp.tile([P, KC, C], BF, tag="w768", name="w768")
                nc.sync.dma_start(
                    wpr[:], wproj_d[l].rearrange("(c p) n -> p c n", p=P)
                )
                for t in TB:
                    ps = pw.tile([P, C], F32, tag="psw", name="psw")
                    for c in range(KC):
                        nc.tensor.matmul(
                            ps[:, 0:512], yT[c][:, t * P : (t + 1) * P],
                            wpr[:, c, 0:512], start=(c == 0), stop=(c == KC - 1),
                        )
                        nc.tensor.matmul(
                            ps[:, 512:768], yT[c][:, t * P : (t + 1) * P],
                            wpr[:, c, 512:768], start=(c == 0), stop=(c == KC - 1),
                        )
                    nc.vector.tensor_tensor(x[t][:], x[t][:], ps[:], op=OP.add)

                # ---- ln2 ----
                layer_norm(TB)

                # ---- mlp: fc+gelu then fcp, in token groups (SBUF economy) ----
                groups = [[7]] if last else [[0, 1, 2, 3], [4, 5, 6, 7]]
                for grp in groups:
                    go, gw = grp[0] * P, len(grp) * P
                    hg = [
                        mp.tile([P, 4 * P], BF, tag=f"h{d}", name=f"h{d}")
                        for d in range(NHID)
                    ]
                    for nB in range(KC):  # 6 column blocks of 512 over HID
                        wf = wfp.tile([P, KC, 512], BF, tag="wf", name="wf")
                        nc.sync.dma_start(
                            wf[:],
                            wfc_d[l, :, nB * 512 : (nB + 1) * 512].rearrange(
                                "(c p) n -> p c n", p=P
                            ),
                        )
                        for dl in range(4):
                            d = nB * 4 + dl
                            ps = pb.tile([P, 512], F32, tag="ps", name="psf")
                            for c in range(KC):
                                nc.tensor.matmul(
                                    ps[:, :gw],
                                    wf[:, c, dl * P : (dl + 1) * P],
                                    xnT[c][:, go : go + gw],
                                    start=(c == 0), stop=(c == KC - 1),
                                )
                            nc.scalar.activation(
                                hg[d][:, :gw], ps[:, :gw], AF.Gelu,
                                bias=zb[:, 0:1],
                            )
                    for cB, co, cw in ((0, 0, 512), (1, 512, 256)):
                        wfcp = wfpp.tile([P, NHID, 512], BF, tag="wfcp", name="wfcp")
                        nc.sync.dma_start(
                            wfcp[:, :, :cw],
                            wfcp_d[l, :, co : co + cw].rearrange(
                                "(k p) n -> p k n", p=P
                            ),
                        )
                        for ti, t in enumerate(grp):
                            ps = pb.tile([P, 512], F32, tag="ps", name="psfp")
                            for k in range(NHID):
                                nc.tensor.matmul(
                                    ps[:, :cw],
                                    hg[k][:, ti * P : (ti + 1) * P],
                                    wfcp[:, k, :cw],
                                    start=(k == 0), stop=(k == NHID - 1),
                                )
                            nc.vector.tensor_tensor(
                                x[t][:, co : co + cw], x[t][:, co : co + cw],
                                ps[:, :cw], op=OP.add,
                            )

            # ---- final LN on block 7, exchange last-token vectors ----
            layer_norm([7], transpose=False)
            nc.sync.dma_start(cc_in[:], x[7][P - 1 : P, :])
            nc.gpsimd.collective_compute(
                "AllGather", OP.bypass,
                replica_groups=[list(range(NCORES))],
                ins=[cc_in[:]], outs=[cc_out[:]],
            )
            xfall = cp.tile([NCORES, C], F32, tag="xfall", name="xfall")
            nc.sync.dma_start(xfall[:], cc_out[:])
            xfbf = cp.tile([NCORES, C], BF, tag="xfbf", name="xfbf")
            nc.vector.tensor_copy(xfbf[:], xfall[:])
            xfT = cp.tile([P, KC, B], BF, tag="xfT", name="xfT")
            for b in range(B):
                for c in range(KC):
                    nc.sync.dma_start(
                        xfT[:, c, b : b + 1],
                        xfbf[2 * b : 2 * b + 1, c * P : (c + 1) * P],
                    )

            # ---- lm head over vocab shard ----
            for nB in range(VS // 512):  # 13
                wl = wp.tile([P, KC, 512], BF, tag="w512", name="w512")
                nc.sync.dma_start(
                    wl[:],
                    wlm_d[:, nB * 512 : (nB + 1) * 512].rearrange(
                        "(c p) n -> p c n", p=P
                    ),
                )
                ps = pb.tile([P, 512], F32, tag="ps", name="psb")
                for c in range(KC):
                    nc.tensor.matmul(
                        ps[:B, :], xfT[:, c, :], wl[:, c, :],
                        start=(c == 0), stop=(c == KC - 1),
                    )
                lmout = sp.tile([B, 512], F32, tag="lmout", name="lmout")
                nc.vector.tensor_copy(lmout[:], ps[:B, :])
                nc.sync.dma_start(
                    logits_d[:, nB * 512 : (nB + 1) * 512], lmout[:]
                )

    split_excess_waits(nc)
    return nc


def _prep(inputs):
    bf = ml_dtypes.bfloat16
    idx = np.asarray(inputs["idx"]).astype(np.int32)  # [B, T]
    wte = np.asarray(inputs["wte"], dtype=np.float32)
    wte_pad = np.zeros((VP, C), dtype=bf)
    wte_pad[:V] = wte.astype(bf)
    wlmT = np.ascontiguousarray(wte_pad.T)  # [C, VP] bf16
    wpe = np.asarray(inputs["wpe"], dtype=np.float32)[:T]
    wqkv = np.asarray(inputs["w_qkv"], dtype=np.float32).copy()
    wqkv[:, :, :C] *= 1.0 / np.sqrt(DH)  # fold attention scale into q
    wqkv = wqkv.astype(bf)
    wproj = np.asarray(inputs["w_proj"], dtype=np.float32).astype(bf)
    wfc = np.asarray(inputs["w_fc"], dtype=np.float32).astype(bf)
    wfcp = np.asarray(inputs["w_fcp"], dtype=np.float32).astype(bf)

    # extended causal mask: masks[kp, m] = 1 iff kp <= m - 384, so the slice
    # at column offset 384-128*j gives the mask for diagonal key-block j
    m = np.arange(896)[None, :]
    kp = np.arange(P)[:, None]
    masks = (kp <= m - 384).astype(bf)
    ones = np.ones((P, P), dtype=bf)

    common = dict(
        wte=wte_pad, wpe=wpe, wqkv=wqkv, wproj=wproj, wfc=wfc, wfcp=wfcp,
        masks=masks, ones=ones, eye=np.eye(P, dtype=bf),
    )
    in_maps = []
    for c in range(NCORES):
        b = c // 2
        m = dict(common)
        m["idx"] = idx[b].reshape(NT, P)
        m["wlm"] = np.ascontiguousarray(wlmT[:, c * VS : (c + 1) * VS])
        in_maps.append(m)
    return in_maps


def kernel(**inputs) -> np.ndarray:
    if "nc" not in _cache:
        _cache["nc"] = _build()
    nc = _cache["nc"]
    in_maps = _prep(inputs)
    res = run_bass_kernel_spmd(nc, in_maps, core_ids=list(range(NCORES)))
    shards = [res.results[c]["logits"] for c in range(NCORES)]
    full = np.concatenate(shards, axis=1)[:, :V]
    return full.reshape(B, 1, V).astype(np.float32)



# revision 20
# speedup vs baseline: 1.3354x; 1.3354x over previous
"""GPT-2-small (B=4,T=1024,C=768,H=12,L=2,V=50257) forward, last-token logits.

Sharding: core c handles batch b=c//2 (body replicated within each core pair)
and vocab shard c of the tied lm_head. One tiny AllGather exchanges the four
final-LN last-token vectors so every core can compute all 4 batches against
its own vocab shard. Matmuls run in bf16 (fp32 accumulate in PSUM).

Layout strategy: activations channel-major ("T" = transposed [C,tok]) feed
the PE as stationary/moving operands without any transposes inside attention;
scores are computed k-major (ST layout) so softmax sums use a ones-matmul and
exp runs on the scalar engine. Only layer-norm outputs are transposed
(PE transpose via identity). Layer 2 computes K/V for all tokens but runs
attention/MLP only for the last 128-token block (logits need only token 1023).
"""
import sys, os
sys.path.insert(0, "/opt/trn_rl_repo")
sys.path.insert(0, os.path.dirname(os.path.abspath(__file__)))
import numpy as np
import ml_dtypes
from concourse import bass, mybir
import concourse.tile as tile
from concourse.bass_utils import run_bass_kernel_spmd

# ---- inlined walrus wait-limit workaround (was tile_patch.py) ----
# The tail drain and DMA pseudo-instructions may carry only ~1 sem-wait each
# under this walrus build; split excess waits onto same-engine NOPs.
import concourse.tile as tile_mod
from concourse.tile import ScopedClock

MAXW = 1



def _patched_drain_and_barrier(self, tick_clock, wait_clock):
    nc = self.nc
    drain_inst = nc.sync.drain()
    wait_clock.add_sem_waits(
        drain_inst.ins, ScopedClock({None: tick_clock.global_clock})
    )
    si = drain_inst.ins.sync_info
    if si is not None and si.on_wait and len(si.on_wait) > MAXW:
        waits = list(si.on_wait)
        drain_inst.ins.sync_info = mybir.SyncInfo(
            on_wait=waits[:MAXW], on_update=list(si.on_update or [])
        )
        rest = waits[MAXW:]
        while rest:
            nop = nc.sync.nop()
            nop.ins.sync_info = mybir.SyncInfo(on_wait=rest[:MAXW], on_update=[])
            rest = rest[MAXW:]

    nc.all_engine_barrier()
    assert self.sems is not None
    popped = nc._tile_sem_poison_stack.pop()
    assert popped is self._sem_poison
    nc.clear_and_free_semaphores(list(self.sems.allocated().values()))
    nc.all_engine_barrier()


tile_mod.TileContext._drain_and_barrier = _patched_drain_and_barrier

_DMA_LIKE = ("DMA", "Collective", "Memset")
_ctr = [0]


def split_excess_waits(nc):
    """Walrus allows only 1 sem-wait on DMA pseudo-instructions and ~8 on
    regular engine instructions. Move excess waits onto same-engine NOPs
    inserted immediately before the offending instruction."""
    import bass_rust

    for f in nc.m.functions:
        for b in f.blocks:
            il = b.instructions
            i = 0
            while i < len(il):
                inst = il[i]
                si = getattr(inst, "sync_info", None)
                waits = list(si.on_wait) if (si is not None and si.on_wait) else []
                opc = str(getattr(inst, "opcode", ""))
                limit = 1 if any(k in opc for k in _DMA_LIKE) else MAXW
                if len(waits) > limit:
                    keep = waits[-limit:] if limit > 0 else []
                    extra = waits[: len(waits) - limit]
                    inst.sync_info = mybir.SyncInfo(
                        on_wait=keep, on_update=list(si.on_update or [])
                    )
                    while extra:
                        chunk, extra = extra[:MAXW], extra[MAXW:]
                        nop = bass_rust.InstNoOp(
                            name=f"wsplit-{_ctr[0]}", ins=[], outs=[]
                        )
                        _ctr[0] += 1
                        nop.engine = inst.engine
                        nop.sync_info = mybir.SyncInfo(on_wait=chunk, on_update=[])
                        il.insert(i, nop)
                        i += 1
                i += 1

P = 128
B, T, C, H, L = 4, 1024, 768, 12, 2
DH = C // H          # 64
HID = 4 * C          # 3072
V = 50257
NT = T // P          # 8 token blocks
KC = C // P          # 6 channel chunks
NHID = HID // P      # 24
VS = 6656            # vocab shard per core (8*6656 = 53248 >= V)
VP = 8 * VS
EPS = 1e-5
BF = mybir.dt.bfloat16
F32 = mybir.dt.float32
FP8 = mybir.dt.float8e4
DR = mybir.MatmulPerfMode.DoubleRow
AF = mybir.ActivationFunctionType
OP = mybir.AluOpType
NCORES = 8
SQKV = 8.0           # fp8 weight scales (keep values out of subnormal range)
SPROJ = 32.0
SFC = 8.0
SFCP = 32.0

_cache = {}


def _build():
    nc = bass.Bass()
    idx_d = nc.dram_tensor("idx", [NT, P], mybir.dt.int32, kind="ExternalInput")
    wte_d = nc.dram_tensor("wte", [VP, C], BF, kind="ExternalInput")
    wlm_d = nc.dram_tensor("wlm", [C, VS], BF, kind="ExternalInput")
    wpe_d = nc.dram_tensor("wpe", [T, C], F32, kind="ExternalInput")
    wqkv_d = nc.dram_tensor("wqkv", [L, C, 3 * C], FP8, kind="ExternalInput")
    wproj_d = nc.dram_tensor("wproj", [L, C, C], FP8, kind="ExternalInput")
    wfc_d = nc.dram_tensor("wfc", [L, C, HID], FP8, kind="ExternalInput")
    wfcp_d = nc.dram_tensor("wfcp", [L, HID, C], FP8, kind="ExternalInput")
    masks_d = nc.dram_tensor("masks", [P, 896], BF, kind="ExternalInput")
    ones_d = nc.dram_tensor("ones", [P, P], BF, kind="ExternalInput")
    eye_d = nc.dram_tensor("eye", [P, P], BF, kind="ExternalInput")
    logits_d = nc.dram_tensor("logits", [B, VS], F32, kind="ExternalOutput")

    cc_in = nc.dram_tensor("cc_in", [1, C], F32)
    cc_out = nc.dram_tensor("cc_out", [NCORES, C], F32, addr_space="Shared")

    with tile.TileContext(nc) as tc:
        with (
            tc.tile_pool(name="const", bufs=1) as cp,
            tc.tile_pool(name="acts", bufs=1) as ap,
            tc.tile_pool(name="wstream", bufs=2) as wp,
            tc.tile_pool(name="wfcol", bufs=2) as wfp,
            tc.tile_pool(name="wfcpcol", bufs=2) as wfpp,
            tc.tile_pool(name="scratch", bufs=2) as sp,
            tc.tile_pool(name="est", bufs=2) as ep,
            tc.tile_pool(name="mlp", bufs=1) as mp,
            tc.tile_pool(name="pb", bufs=4, space="PSUM") as pb,
            tc.tile_pool(name="pw", bufs=2, space="PSUM") as pw,
        ):
            ones = cp.tile([P, P], BF, tag="ones", name="ones")
            zb = cp.tile([P, 1], F32, tag="zb", name="zb")
            nc.gpsimd.memset(zb[:], 0.0)
            eb = cp.tile([P, 1], F32, tag="eb", name="eb")
            nc.gpsimd.memset(eb[:], EPS)
            nc.sync.dma_start(ones[:], ones_d[:])
            eye = cp.tile([P, P], BF, tag="eye", name="eye")
            nc.sync.dma_start(eye[:], eye_d[:])
            masks = cp.tile([P, 896], BF, tag="masks", name="masks")
            nc.sync.dma_start(masks[:], masks_d[:])
            idx_sb = cp.tile([P, NT], mybir.dt.int32, tag="idx", name="idx")
            nc.sync.dma_start(idx_sb[:], idx_d[:].rearrange("t p -> p t"))

            # persistent activation tiles
            x = [ap.tile([P, C], F32, tag=f"x{t}", name=f"x{t}") for t in range(NT)]
            xq = ap.tile([P, KC, T], FP8, tag="xq", name="xq")
            qkT = [ap.tile([P, T], BF, tag=f"qkT{d}", name=f"qkT{d}") for d in range(12)]
            vtok = [ap.tile([P, C], BF, tag=f"v{t}", name=f"v{t}") for t in range(NT)]
            yq = ap.tile([P, KC, T], FP8, tag="yq", name="yq")
            s1 = ap.tile([P, NT], F32, tag="s1", name="s1")
            s2 = ap.tile([P, NT], F32, tag="s2", name="s2")
            mean = ap.tile([P, NT], F32, tag="mean", name="mean")
            var = ap.tile([P, NT], F32, tag="var", name="var")
            rstd = ap.tile([P, NT], F32, tag="rstd", name="rstd")

            def layer_norm(blocks, transpose=True):
                """x[t] (f32) -> normalized in place; transposed bf16 -> xnT."""
                for t in blocks:
                    nc.vector.reduce_sum(
                        out=s1[:, t : t + 1], in_=x[t][:], axis=mybir.AxisListType.X
                    )
                    sq = sp.tile([P, C], F32, tag="lnsq", name="lnsq")
                    nc.scalar.activation(
                        sq[:], x[t][:], AF.Square, bias=zb[:, 0:1], accum_out=s2[:, t : t + 1]
                    )
                # stats for all blocks at once (extra cols harmless)
                nc.vector.tensor_scalar_mul(mean[:], s1[:], 1.0 / C)
                nc.vector.tensor_scalar_mul(var[:], s2[:], 1.0 / C)
                msq = sp.tile([P, NT], F32, tag="lnmsq", name="lnmsq")
                nc.vector.tensor_tensor(msq[:], mean[:], mean[:], op=OP.mult)
                nc.vector.tensor_tensor(var[:], var[:], msq[:], op=OP.subtract)
                std = sp.tile([P, NT], F32, tag="lnstd", name="lnstd")
                nc.scalar.activation(std[:], var[:], AF.Sqrt, bias=eb[:, 0:1])
                nc.vector.reciprocal(rstd[:], std[:])
                for t in blocks:
                    nc.vector.tensor_scalar(
                        out=x[t][:], in0=x[t][:],
                        scalar1=mean[:, t : t + 1], scalar2=rstd[:, t : t + 1],
                        op0=OP.subtract, op1=OP.mult,
                    )
                    if not transpose:
                        continue
                    xtmp = sp.tile([P, C], BF, tag="xtmp", name="xtmp")
                    nc.vector.tensor_copy(out=xtmp[:], in_=x[t][:])
                    for c in range(KC):
                        tp = pb.tile([P, P], BF, tag="ps", name="tp")
                        nc.tensor.transpose(
                            tp[:], xtmp[:, c * P : (c + 1) * P], eye[:]
                        )
                        nc.vector.tensor_copy(
                            out=xq[:, c, t * P : (t + 1) * P], in_=tp[:]
                        )

            # ---- embedding ----
            for t in range(NT):
                g = sp.tile([P, C], BF, tag="emb", name="emb")
                nc.gpsimd.indirect_dma_start(
                    out=g[:], out_offset=None, in_=wte_d[:],
                    in_offset=bass.IndirectOffsetOnAxis(ap=idx_sb[:, t : t + 1], axis=0),
                )
                pe = sp.tile([P, C], F32, tag="wpe", name="wpe")
                nc.sync.dma_start(pe[:], wpe_d[t * P : (t + 1) * P, :])
                nc.vector.tensor_tensor(x[t][:], g[:], pe[:], op=OP.add)

            for l in range(L):
                last = l == L - 1
                TB = [7] if last else list(range(NT))

                layer_norm(range(NT))

                # ---- q/k channel-major, fp8 DoubleRow ----
                # (layer2: q only for last block)
                for nB in range(3):  # qkv columns [nB*512, nB*512+512)
                    wcol = wp.tile([P, KC, 512], FP8, tag="w512f8", name="wqk")
                    nc.sync.dma_start(
                        wcol[:],
                        wqkv_d[l, :, nB * 512 : (nB + 1) * 512].rearrange(
                            "(c p) n -> p c n", p=P
                        ),
                    )
                    for dl in range(4):
                        d = nB * 4 + dl
                        is_q = d < 6
                        usc = 1.0 / SQKV
                        if last and is_q:
                            spans = [(7 * P, P)]  # q only for token block 7
                        else:
                            spans = [(o, 256) for o in range(0, T, 256)]
                        for (o, w) in spans:
                            ps = pb.tile([64, 512], F32, tag="ps", name="psb")
                            for m in range(2):
                                mo = dl * P + m * 64
                                for j in range(KC // 2):
                                    nc.tensor.matmul(
                                        ps[:, m * w : m * w + w],
                                        wcol[:, 2 * j : 2 * j + 2, mo : mo + 64],
                                        xq[:, 2 * j : 2 * j + 2, o : o + w],
                                        start=(j == 0), stop=(j == 2),
                                        perf_mode=DR,
                                    )
                            nc.vector.tensor_scalar_mul(
                                qkT[d][0:64, o : o + w], ps[:, 0:w], usc
                            )
                            nc.vector.tensor_scalar_mul(
                                qkT[d][64:128, o : o + w], ps[:, w : 2 * w], usc
                            )

                # ---- v token-major, fp8 DoubleRow ----
                wv = wp.tile([P, KC, C], FP8, tag="w768f8", name="wv")
                nc.sync.dma_start(
                    wv[:], wqkv_d[l, :, 2 * C :].rearrange("(c p) n -> p c n", p=P)
                )
                for t in range(NT):
                    ps = pw.tile([P, C], F32, tag="psw", name="psw")
                    for c in range(KC):
                        nc.tensor.matmul(
                            ps[:, 0:512], xq[:, c, t * P : (t + 1) * P],
                            wv[:, c, 0:512], start=(c == 0), stop=(c == KC - 1),
                        )
                        nc.tensor.matmul(
                            ps[:, 512:768], xq[:, c, t * P : (t + 1) * P],
                            wv[:, c, 512:768], start=(c == 0), stop=(c == KC - 1),
                        )
                    nc.vector.tensor_scalar_mul(vtok[t][:], ps[:], 1.0 / SQKV)

                # ---- attention ----
                if last:
                    qspans = [(7 * P, P, 8)]  # (col offset, width, n key blocks)
                else:
                    qspans = [(0, 512, 4), (512, 512, 8)]
                for h in range(H):
                    kt = qkT[6 + h // 2]
                    qt = qkT[h // 2]
                    po = (h % 2) * DH
                    for (qo, qw, nkb) in qspans:
                        den = pb.tile([P, 512], F32, tag="ps", name="den")
                        yp = pb.tile([P, 512], F32, tag="ps", name="yp")
                        for kb in range(nkb):
                            st = pb.tile([P, 512], F32, tag="ps", name="st")
                            nc.tensor.matmul(
                                st[:, :qw],
                                kt[po : po + DH, kb * P : (kb + 1) * P],
                                qt[po : po + DH, qo : qo + qw],
                                start=True, stop=True,
                            )
                            est = ep.tile([P, 512], BF, tag="est", name="est")
                            nc.scalar.activation(est[:, :qw], st[:, :qw], AF.Exp, bias=zb[:, 0:1])
                            # causal mask on diagonal-region key blocks.
                            # query chunk starts at token qo; key block kb is
                            # fully valid iff kb*128+127 <= qo+col for all cols,
                            # i.e. kb < qo//128. Otherwise mask j = kb - qo//128
                            # (mask j: valid iff 128*j + kp <= qcol).
                            jj = kb - qo // P
                            if jj >= 0:
                                mo = 384 - 128 * jj
                                nc.vector.tensor_tensor(
                                    est[:, :qw], est[:, :qw],
                                    masks[:, mo : mo + qw], op=OP.mult,
                                )
                            nc.tensor.matmul(
                                den[:, :qw], ones[:], est[:, :qw],
                                start=(kb == 0), stop=(kb == nkb - 1),
                            )
                            nc.tensor.matmul(
                                yp[:DH, :qw], vtok[kb][:, h * DH : (h + 1) * DH],
                                est[:, :qw], start=(kb == 0), stop=(kb == nkb - 1),
                            )
                        rec = ep.tile([DH, 512], F32, tag="rec", name="rec")
                        nc.vector.reciprocal(rec[:, :qw], den[:DH, :qw])
                        nc.vector.tensor_tensor(
                            yq[po : po + DH, h // 2, qo : qo + qw],
                            yp[:DH, :qw], rec[:, :qw], op=OP.mult,
                        )

                # ---- attn proj + residual, fp8 DoubleRow ----
                wpr = wp.tile([P, KC, C], FP8, tag="w768f8", name="wpr")
                nc.sync.dma_start(
                    wpr[:], wproj_d[l].rearrange("(c p) n -> p c n", p=P)
                )
                for t in TB:
                    ps = pw.tile([P, C], F32, tag="psw", name="psw")
                    for c in range(KC):
                        nc.tensor.matmul(
                            ps[:, 0:512], yq[:, c, t * P : (t + 1) * P],
                            wpr[:, c, 0:512], start=(c == 0), stop=(c == KC - 1),
                        )
                        nc.tensor.matmul(
                            ps[:, 512:768], yq[:, c, t * P : (t + 1) * P],
                            wpr[:, c, 512:768], start=(c == 0), stop=(c == KC - 1),
                        )
                    nc.vector.scalar_tensor_tensor(
                        out=x[t][:], in0=ps[:], scalar=1.0 / SPROJ, in1=x[t][:],
                        op0=OP.mult, op1=OP.add,
                    )

                # ---- ln2 ----
                layer_norm(TB)

                # ---- mlp: fc+gelu (fp8 DR) into hq, then fcp (fp8 DR) ----
                fc_spans = (
                    [(7 * P, P)] if last else [(o, 256) for o in range(0, T, 256)]
                )
                hq = mp.tile([P, NHID, T], FP8, tag="hq", name="hq")
                for nB in range(KC):  # 6 column blocks of 512 over HID
                    wf = wfp.tile([P, KC, 512], FP8, tag="wf", name="wf")
                    nc.sync.dma_start(
                        wf[:],
                        wfc_d[l, :, nB * 512 : (nB + 1) * 512].rearrange(
                            "(c p) n -> p c n", p=P
                        ),
                    )
                    for dl in range(4):
                        d = nB * 4 + dl
                        for (o, w) in fc_spans:
                            ps = pb.tile([64, 512], F32, tag="ps", name="psf")
                            for m in range(2):
                                mo = dl * P + m * 64
                                for j in range(KC // 2):
                                    nc.tensor.matmul(
                                        ps[:, m * w : m * w + w],
                                        wf[:, 2 * j : 2 * j + 2, mo : mo + 64],
                                        xq[:, 2 * j : 2 * j + 2, o : o + w],
                                        start=(j == 0), stop=(j == 2),
                                        perf_mode=DR,
                                    )
                            nc.scalar.activation(
                                hq[0:64, d, o : o + w], ps[:, 0:w], AF.Gelu,
                                bias=zb[0:64, 0:1], scale=1.0 / SFC,
                            )
                            nc.scalar.activation(
                                hq[64:128, d, o : o + w], ps[:, w : 2 * w], AF.Gelu,
                                bias=zb[0:64, 0:1], scale=1.0 / SFC,
                            )
                for cB in range(3):  # 3 column blocks of 256 over C
                    co = cB * 256
                    wfcp = wfpp.tile([P, NHID, 256], FP8, tag="wfcp", name="wfcp")
                    nc.sync.dma_start(
                        wfcp[:],
                        wfcp_d[l, :, co : co + 256].rearrange(
                            "(k p) n -> p k n", p=P
                        ),
                    )
                    for t in TB:
                        ps = pb.tile([64, 512], F32, tag="ps", name="psfp")
                        for m in range(2):
                            to = t * P + m * 64
                            for k in range(NHID // 2):
                                nc.tensor.matmul(
                                    ps[:, m * 256 : m * 256 + 256],
                                    hq[:, 2 * k : 2 * k + 2, to : to + 64],
                                    wfcp[:, 2 * k : 2 * k + 2, :],
                                    start=(k == 0), stop=(k == NHID // 2 - 1),
                                    perf_mode=DR,
                                )
                        for m in range(2):
                            nc.vector.scalar_tensor_tensor(
                                out=x[t][m * 64 : m * 64 + 64, co : co + 256],
                                in0=ps[:, m * 256 : m * 256 + 256],
                                scalar=1.0 / SFCP,
                                in1=x[t][m * 64 : m * 64 + 64, co : co + 256],
                                op0=OP.mult, op1=OP.add,
                            )

            # ---- final LN on block 7, exchange last-token vectors ----
            layer_norm([7], transpose=False)
            nc.sync.dma_start(cc_in[:], x[7][P - 1 : P, :])
            nc.gpsimd.collective_compute(
                "AllGather", OP.bypass,
                replica_groups=[list(range(NCORES))],
                ins=[cc_in[:]], outs=[cc_out[:]],
            )
            xfall = cp.tile([NCORES, C], F32, tag="xfall", name="xfall")
            nc.sync.dma_start(xfall[:], cc_out[:])
            xfbf = cp.tile([NCORES, C], BF, tag="xfbf", name="xfbf")
            nc.vector.tensor_copy(xfbf[:], xfall[:])
            xfT = cp.tile([P, KC, B], BF, tag="xfT", name="xfT")
            for b in range(B):
                for c in range(KC):
                    nc.sync.dma_start(
                        xfT[:, c, b : b + 1],
                        xfbf[2 * b : 2 * b + 1, c * P : (c + 1) * P],
                    )

            # ---- lm head over vocab shard ----
            for nB in range(VS // 512):  # 13
                wl = wp.tile([P, KC, 512], BF, tag="w512", name="w512")
                nc.sync.dma_start(
                    wl[:],
                    wlm_d[:, nB * 512 : (nB + 1) * 512].rearrange(
                        "(c p) n -> p c n", p=P
                    ),
                )
                ps = pb.tile([P, 512], F32, tag="ps", name="psb")
                for c in range(KC):
                    nc.tensor.matmul(
                        ps[:B, :], xfT[:, c, :], wl[:, c, :],
                        start=(c == 0), stop=(c == KC - 1),
                    )
                lmout = sp.tile([B, 512], F32, tag="lmout", name="lmout")
                nc.vector.tensor_copy(lmout[:], ps[:B, :])
                nc.sync.dma_start(
                    logits_d[:, nB * 512 : (nB + 1) * 512], lmout[:]
                )

    split_excess_waits(nc)
    return nc


def _prep(inputs):
    bf = ml_dtypes.bfloat16
    f8 = ml_dtypes.float8_e4m3fn
    idx = np.asarray(inputs["idx"]).astype(np.int32)  # [B, T]
    wte = np.asarray(inputs["wte"], dtype=np.float32)
    wte_pad = np.zeros((VP, C), dtype=bf)
    wte_pad[:V] = wte.astype(bf)
    wlmT = np.ascontiguousarray(wte_pad.T)  # [C, VP] bf16
    wpe = np.asarray(inputs["wpe"], dtype=np.float32)[:T]
    wqkv = np.asarray(inputs["w_qkv"], dtype=np.float32).copy()
    wqkv[:, :, :C] *= 1.0 / np.sqrt(DH)  # fold attention scale into q
    wqkv = (wqkv * SQKV).astype(f8)
    wproj = (np.asarray(inputs["w_proj"], dtype=np.float32) * SPROJ).astype(f8)
    wfc = (np.asarray(inputs["w_fc"], dtype=np.float32) * SFC).astype(f8)
    wfcp = (np.asarray(inputs["w_fcp"], dtype=np.float32) * SFCP).astype(f8)

    # extended causal mask: masks[kp, m] = 1 iff kp <= m - 384, so the slice
    # at column offset 384-128*j gives the mask for diagonal key-block j
    m = np.arange(896)[None, :]
    kp = np.arange(P)[:, None]
    masks = (kp <= m - 384).astype(bf)
    ones = np.ones((P, P), dtype=bf)

    common = dict(
        wte=wte_pad, wpe=wpe, wqkv=wqkv, wproj=wproj, wfc=wfc, wfcp=wfcp,
        masks=masks, ones=ones, eye=np.eye(P, dtype=bf),
    )
    in_maps = []
    for c in range(NCORES):
        b = c // 2
        m = dict(common)
        m["idx"] = idx[b].reshape(NT, P)
        m["wlm"] = np.ascontiguousarray(wlmT[:, c * VS : (c + 1) * VS])
        in_maps.append(m)
    return in_maps


def kernel(**inputs) -> np.ndarray:
    if "nc" not in _cache:
        _cache["nc"] = _build()
    nc = _cache["nc"]
    in_maps = _prep(inputs)
    res = run_bass_kernel_spmd(nc, in_maps, core_ids=list(range(NCORES)))
    shards = [res.results[c]["logits"] for c in range(NCORES)]
    full = np.concatenate(shards, axis=1)[:, :V]
    return full.reshape(B, 1, V).astype(np.float32)

